# revision 5
# baseline (speedup 1.0000x reference)
"""DAGCN layer kernel for 8 Trainium2 NeuronCores (Bass/Tile, SPMD).

Math (equal to the reference by linearity of the edge MLP):
  hs = h @ W_src ; ht = h @ W_tgt
  agg[n] = (sum_{e:dst=n} hs[src[e]] + deg[n]*(ht[n] + b_src + b_tgt)) / max(deg[n],1)
  then multi-head attention (q from h rows, k/v from agg) + FFN with LayerNorms.

Sharding: core c owns dst/query rows [512c, 512c+512). The segment-sum over
edges is a dense matmul against the per-core adjacency count matrix
A_T[src, dst_local] (built host-side from edge_index; small ints, exact in
fp8):  aggT = hs^T @ A_T  in fp8 DoubleRow (2 contraction chunks/pass).
agg slices are AllGathered in fp8 so each core holds full k/v. The attention
ctx matmul also runs fp8 DoubleRow; scores stay bf16. b_v is folded into b_o
host-side (a constant added to v shifts ctx by that constant).
"""

import contextlib
import numpy as np
import ml_dtypes

import concourse.bass as bass
import concourse.bacc as bacc
import concourse.tile as tile
from concourse import mybir
from concourse.bass_utils import run_bass_kernel_spmd
from concourse.masks import make_identity

N, H, HEADS, E = 4096, 256, 4, 262144
DH = H // HEADS          # 64
NCORES = 8
RPC = N // NCORES        # 512 nodes per core
NBLK = RPC // 128        # 4 dst blocks per core
EPS = 1e-5
NCH = N // 128           # 32
KCP = NCH // 2           # 16 chunk pairs
HCH = H // 128           # 2

BF = mybir.dt.bfloat16
F32 = mybir.dt.float32
F8 = mybir.dt.float8e4
DR = mybir.MatmulPerfMode.DoubleRow
AF = mybir.ActivationFunctionType

_CACHE = {}


def _layernorm_rows(nc, pool, z, out_t, g, be, i, nm, epst=None):
    """LayerNorm along the free dim of a [128, H] f32 row tile."""
    stats = pool.tile([128, 6], F32, name=f"{nm}_st{i}", tag=f"{nm}_st")
    nc.vector.bn_stats(out=stats[:], in_=z[:])
    mv = pool.tile([128, 2], F32, name=f"{nm}_mv{i}", tag=f"{nm}_mv")
    nc.vector.bn_aggr(out=mv[:], in_=stats[:])
    sd = pool.tile([128, 1], F32, name=f"{nm}_sd{i}", tag=f"{nm}_sd")
    nc.scalar.activation(out=sd[:], in_=mv[:, 1:2], func=AF.Sqrt,
                         bias=epst[:, 0:1], scale=1.0)
    rstd = pool.tile([128, 1], F32, name=f"{nm}_rs{i}", tag=f"{nm}_rs")
    nc.vector.reciprocal(out=rstd[:], in_=sd[:])
    nmu = pool.tile([128, 1], F32, name=f"{nm}_nm{i}", tag=f"{nm}_nm")
    nc.vector.tensor_tensor(out=nmu[:], in0=mv[:, 0:1], in1=rstd[:],
                            op=mybir.AluOpType.mult)
    nc.vector.tensor_scalar_mul(nmu[:], nmu[:], -1.0)
    zn = pool.tile([128, z.shape[1]], F32, name=f"{nm}_zn{i}", tag=f"{nm}_zn")
    nc.scalar.activation(out=zn[:], in_=z[:], func=AF.Identity,
                         bias=nmu[:, 0:1], scale=rstd[:, 0:1])
    nc.vector.tensor_tensor(out=zn[:], in0=zn[:], in1=g[:], op=mybir.AluOpType.mult)
    nc.vector.tensor_add(out_t[:], zn[:], be[:])


def _build_program():
    nc = bacc.Bacc("TRN2", target_bir_lowering=False, debug=False, num_devices=NCORES)

    # fp8 inputs: feature-dim-interleaved pairs for DoubleRow matmuls
    hT8_d = nc.dram_tensor("hT8", [128, HCH * N], F8, kind="ExternalInput")
    hTo8_d = nc.dram_tensor("hTo8", [128, HCH * RPC], F8, kind="ExternalInput")
    at8_d = nc.dram_tensor("at8", [16 * 128, 2 * RPC], F8, kind="ExternalInput")
    ws8_d = nc.dram_tensor("ws8", [128, HCH * H], F8, kind="ExternalInput")
    wt8_d = nc.dram_tensor("wt8", [128, HCH * H], F8, kind="ExternalInput")
    wq8_d = nc.dram_tensor("wq8", [128, HCH * H], F8, kind="ExternalInput")
    wk8_d = nc.dram_tensor("wk8", [128, HCH * H], F8, kind="ExternalInput")
    wv8_d = nc.dram_tensor("wv8", [128, HCH * H], F8, kind="ExternalInput")
    h_rows = nc.dram_tensor("h_rows", [RPC, H], F32, kind="ExternalInput")
    rw_d = nc.dram_tensor("rw_b", [128, 2 * RPC], F32, kind="ExternalInput")
    w_o = nc.dram_tensor("w_o", [H, H], BF, kind="ExternalInput")
    w_1 = nc.dram_tensor("w_1", [H, 2 * H], BF, kind="ExternalInput")
    w_2 = nc.dram_tensor("w_2", [2 * H, H], BF, kind="ExternalInput")
    bst_c = nc.dram_tensor("bst_c", [128, HCH], F32, kind="ExternalInput")
    bq_c = nc.dram_tensor("bq_c", [128, HCH], F32, kind="ExternalInput")
    bk_c = nc.dram_tensor("bk_c", [128, HCH], F32, kind="ExternalInput")
    bo_b = nc.dram_tensor("bo_b", [128, H], F32, kind="ExternalInput")
    b1_c = nc.dram_tensor("b1_c", [128, 4], F32, kind="ExternalInput")
    b2_b = nc.dram_tensor("b2_b", [128, H], F32, kind="ExternalInput")
    g1_b = nc.dram_tensor("g1_b", [128, H], F32, kind="ExternalInput")
    be1_b = nc.dram_tensor("be1_b", [128, H], F32, kind="ExternalInput")
    g2_b = nc.dram_tensor("g2_b", [128, H], F32, kind="ExternalInput")
    be2_b = nc.dram_tensor("be2_b", [128, H], F32, kind="ExternalInput")
    out = nc.dram_tensor("out", [RPC, H], F32, kind="ExternalOutput")

    with tile.TileContext(nc) as tc, contextlib.ExitStack() as ctx:
        singles = ctx.enter_context(tc.tile_pool(name="singles", bufs=1))
        apool = ctx.enter_context(tc.tile_pool(name="apool", bufs=1))
        hspool = ctx.enter_context(tc.tile_pool(name="hspool", bufs=1))
        epool = ctx.enter_context(tc.tile_pool(name="epool", bufs=4))

        # ---------- constants ----------
        At = [apool.tile([128, 2, RPC], F8, name=f"At{t}") for t in range(16)]
        for t in range(16):
            nc.sync.dma_start(out=At[t][:], in_=at8_d[t * 128:(t + 1) * 128, :])
        hT8 = singles.tile([128, HCH, N], F8, name="hT8")
        nc.sync.dma_start(out=hT8[:], in_=hT8_d[:])
        hTo8 = singles.tile([128, HCH, RPC], F8, name="hTo8")
        nc.sync.dma_start(out=hTo8[:], in_=hTo8_d[:])

        def load8(t, name):
            w = singles.tile([128, HCH, H], F8, name=name)
            nc.sync.dma_start(out=w[:], in_=t[:])
            return w

        Ws8 = load8(ws8_d, "Ws8")
        Wt8 = load8(wt8_d, "Wt8")
        Wq8 = load8(wq8_d, "Wq8")
        Wk8 = load8(wk8_d, "Wk8")
        Wv8 = load8(wv8_d, "Wv8")

        def load_w(t, name, rows, cols):
            w = [singles.tile([128, cols], BF, name=f"{name}{i}") for i in range(rows // 128)]
            for i in range(rows // 128):
                nc.sync.dma_start(out=w[i][:], in_=t[i * 128:(i + 1) * 128, :])
            return w

        Wo = load_w(w_o, "Wo", H, H)
        W1 = load_w(w_1, "W1", H, 2 * H)
        W2 = load_w(w_2, "W2", 2 * H, H)

        def load_b(t, name, shape):
            b = singles.tile(list(shape), F32, name=name)
            nc.sync.dma_start(out=b[:], in_=t[:])
            return b

        rwb = load_b(rw_d, "rwb", (128, 2 * RPC))
        bstc = load_b(bst_c, "bstc", (128, HCH))
        bqc = load_b(bq_c, "bqc", (128, HCH))
        bkc = load_b(bk_c, "bkc", (128, HCH))
        bob = load_b(bo_b, "bob", (128, H))
        b1c = load_b(b1_c, "b1c", (128, 4))
        b2b = load_b(b2_b, "b2b", (128, H))
        g1b = load_b(g1_b, "g1b", (128, H))
        be1b = load_b(be1_b, "be1b", (128, H))
        g2b = load_b(g2_b, "g2b", (128, H))
        be2b = load_b(be2_b, "be2b", (128, H))

        hrows = [singles.tile([128, H], F32, name=f"hrows{i}") for i in range(NBLK)]
        for i in range(NBLK):
            nc.sync.dma_start(out=hrows[i][:], in_=h_rows[i * 128:(i + 1) * 128, :])
        ident = singles.tile([128, 128], F32)
        make_identity(nc, ident[:])
        epst = singles.tile([128, 1], F32)
        nc.vector.memset(epst[:], EPS)

        # ---------- phase 1+2: hs chunks + aggT = hs^T @ A_T (fp8 DoubleRow) ----
        ph1 = tc.tile_pool(name="ph1_ps", bufs=2, space="PSUM")
        hs_ps = ph1.__enter__()
        ph2 = tc.tile_pool(name="agg_ps", bufs=1, space="PSUM")
        agg_ps = ph2.__enter__()
        mT = [agg_ps.tile([128, RPC], F32, name=f"mT{j}") for j in range(HCH)]
        for t in range(16):
            hsp = hspool.tile([128, 2, H], F8, name=f"hs{t}")
            for i in range(2):
                ps = hs_ps.tile([128, H], F32)
                nc.tensor.matmul(out=ps[:], lhsT=hT8[:, :, (2 * t + i) * 128:(2 * t + i + 1) * 128],
                                 rhs=Ws8[:], start=True, stop=True, perf_mode=DR)
                nc.vector.tensor_copy(out=hsp[:, i, :], in_=ps[:])
            for j in range(HCH):
                nc.tensor.matmul(out=mT[j][:], lhsT=hsp[:, :, j * 128:(j + 1) * 128],
                                 rhs=At[t][:], start=(t == 0), stop=(t == 15),
                                 perf_mode=DR)

        # htT[j] = (Wtgt^T @ h_own^T)[j]  -> [128 feat, RPC]
        htT_ps = [agg_ps.tile([128, RPC], F32, name=f"htTp{j}") for j in range(HCH)]
        for j in range(HCH):
            nc.tensor.matmul(out=htT_ps[j][:], lhsT=Wt8[:, :, j * 128:(j + 1) * 128],
                             rhs=hTo8[:], start=True, stop=True, perf_mode=DR)

        # finalize: aggT = mT * recB + wB * (htT + bst)   (fp8 out)
        aggT_own = [singles.tile([128, RPC], F8, name=f"aggTo{j}") for j in range(HCH)]
        for j in range(HCH):
            t1 = singles.tile([128, RPC], F32, name=f"fin1_{j}", tag="fin1")
            nc.scalar.activation(out=t1[:], in_=htT_ps[j][:], func=AF.Identity,
                                 bias=bstc[:, j:j + 1], scale=1.0)
            nc.vector.tensor_tensor(out=t1[:], in0=t1[:], in1=rwb[:, RPC:2 * RPC],
                                    op=mybir.AluOpType.mult)
            t2 = singles.tile([128, RPC], F32, name=f"fin2_{j}", tag="fin2")
            nc.vector.tensor_tensor(out=t2[:], in0=mT[j][:], in1=rwb[:, 0:RPC],
                                    op=mybir.AluOpType.mult)
            nc.vector.tensor_add(aggT_own[j][:], t1[:], t2[:])

        ph2.__exit__(None, None, None)
        ph1.__exit__(None, None, None)

        # ---------- phase 3: AllGather (fp8 payload) ----------
        with tc.tile_pool(name="dram", bufs=1, space="DRAM") as dram:
            cc_in = dram.tile([H, RPC], F8)
            cc_out = dram.tile([NCORES * H, RPC], F8, addr_space="Shared")
            for j in range(HCH):
                nc.gpsimd.dma_start(out=cc_in[j * 128:(j + 1) * 128, :], in_=aggT_own[j][:])
            nc.gpsimd.collective_compute(
                "AllGather", mybir.AluOpType.bypass,
                replica_groups=[list(range(NCORES))],
                ins=[cc_in.opt()], outs=[cc_out.opt()])

            # qT (overlaps the collective)
            ph3 = tc.tile_pool(name="kvq_ps", bufs=2, space="PSUM")
            mid_ps = ph3.__enter__()
            qT = [singles.tile([128, RPC], BF, name=f"qT{j}") for j in range(HCH)]
            for j in range(HCH):
                ps = mid_ps.tile([128, RPC], F32, name=f"qTp{j}", tag="kvq")
                nc.tensor.matmul(out=ps[:], lhsT=Wq8[:, :, j * 128:(j + 1) * 128],
                                 rhs=hTo8[:], start=True, stop=True, perf_mode=DR)
                nc.scalar.activation(out=qT[j][:], in_=ps[:], func=AF.Identity,
                                     bias=bqc[:, j:j + 1], scale=1.0)

            # vext ones columns (constant, set once)
            vext = [singles.tile([128, 2, 272], F8, name=f"vext{p}")
                    for p in range(KCP)]
            for p in range(KCP):
                for i in range(2):
                    for h in range(HEADS):
                        nc.vector.memset(vext[p][:, i, h * (DH + 1) + DH:(h + 1) * (DH + 1)], 1.0)

            # gathered aggT, regrouped per source core, j-interleaved
            aggTc = [singles.tile([128, 2, RPC], F8, name=f"aggTc{c}") for c in range(NCORES)]
            ccv = cc_out[:].rearrange("(c h) f -> c h f", c=NCORES)
            for c in range(NCORES):
                for j in range(HCH):
                    nc.sync.dma_start(out=aggTc[c][:, j, :],
                                      in_=ccv[c, j * 128:(j + 1) * 128, :])

            # ---------- phase 4: kT (bf16 out) and v_ext (fp8) ----------
            kT = [singles.tile([128, N], BF, name=f"kT{j}") for j in range(HCH)]
            for j in range(HCH):
                for c in range(NCORES):
                    ps = mid_ps.tile([128, RPC], F32, name=f"kTp{j}_{c}", tag="kvq")
                    nc.tensor.matmul(out=ps[:], lhsT=Wk8[:, :, j * 128:(j + 1) * 128],
                                     rhs=aggTc[c][:], start=True, stop=True, perf_mode=DR)
                    nc.scalar.activation(out=kT[j][:, c * RPC:(c + 1) * RPC],
                                         in_=ps[:], func=AF.Identity,
                                         bias=bkc[:, j:j + 1], scale=1.0)
            for kc in range(NCH):
                c, b = kc // NBLK, kc % NBLK
                ps = mid_ps.tile([128, H], F32, name=f"vp{kc}", tag="vp")
                nc.tensor.matmul(out=ps[:], lhsT=aggTc[c][:, :, b * 128:(b + 1) * 128],
                                 rhs=Wv8[:], start=True, stop=True, perf_mode=DR)
                for h in range(HEADS):
                    nc.vector.tensor_copy(
                        out=vext[kc // 2][:, kc % 2, h * (DH + 1):h * (DH + 1) + DH],
                        in_=ps[:, h * DH:(h + 1) * DH])

            ph3.__exit__(None, None, None)

            # ---------- phase 5: attention ----------
            SCALE = float(1.0 / np.sqrt(DH))
            ph5c = tc.tile_pool(name="ctx_ps", bufs=1, space="PSUM")
            ctx_ps = ph5c.__enter__()
            ph5q = tc.tile_pool(name="qk_ps", bufs=2, space="PSUM")
            qk_ps = ph5q.__enter__()
            ctxp = [ctx_ps.tile([DH + 1, RPC], F32, name=f"ctxp{h}") for h in range(HEADS)]
            for p in range(KCP):
                for h in range(HEADS):
                    j, r = h // 2, (h % 2) * 64
                    sc = qk_ps.tile([128, 2 * RPC], F32, name=f"sc{h}_{p}", tag="sc")
                    for i in range(2):
                        kc = 2 * p + i
                        nc.tensor.matmul(out=sc[:, i * RPC:(i + 1) * RPC],
                                         lhsT=kT[j][r:r + 64, kc * 128:(kc + 1) * 128],
                                         rhs=qT[j][r:r + 64, :], start=True, stop=True,
                                         tile_position=(r, 0))
                    e = epool.tile([128, 2, RPC], F8, name=f"e{h}_{p}", tag=f"e{h}")
                    nc.scalar.activation(out=e[:, :, :], in_=sc[:], func=AF.Exp, scale=SCALE)
                    nc.tensor.matmul(out=ctxp[h][:],
                                     lhsT=vext[p][:, :, h * (DH + 1):(h + 1) * (DH + 1)],
                                     rhs=e[:, :, :], start=(p == 0), stop=(p == KCP - 1),
                                     perf_mode=DR)

            ph5q.__exit__(None, None, None)
            ph5b = tc.tile_pool(name="bc_ps", bufs=1, space="PSUM")
            bc_ps = ph5b.__enter__()
            ctxT = [singles.tile([128, RPC], BF, name=f"ctxT{j}") for j in range(HCH)]
            ones1 = singles.tile([1, DH], F32)
            nc.vector.memset(ones1[:], 1.0)
            for h in range(HEADS):
                rs = singles.tile([1, RPC], F32, name=f"rs{h}", tag="rs")
                nc.vector.tensor_copy(out=rs[:], in_=ctxp[h][DH:DH + 1, :])
                rrec = singles.tile([1, RPC], F32, name=f"rrec{h}", tag="rrec")
                nc.vector.reciprocal(out=rrec[:], in_=rs[:])
                bc = bc_ps.tile([DH, RPC], F32, name=f"bc{h}", tag="bc")
                nc.tensor.matmul(out=bc[:], lhsT=ones1[:], rhs=rrec[:], start=True, stop=True)
                cs = singles.tile([DH, RPC], F32, name=f"cs{h}", tag="cs")
                nc.vector.tensor_copy(out=cs[:], in_=bc[:])
                j, r = h // 2, (h % 2) * 64
                nc.vector.tensor_tensor(out=ctxT[j][r:r + 64, :], in0=ctxp[h][0:DH, :],
                                        in1=cs[:], op=mybir.AluOpType.mult)

            ph5b.__exit__(None, None, None)
            ph5c.__exit__(None, None, None)
            ph6 = tc.tile_pool(name="fin_ps", bufs=2, space="PSUM")
            mid_ps = ph6.__enter__()

            # attn_out rows + residual + LN1
            xrows = [singles.tile([128, H], F32, name=f"xrows{i}") for i in range(NBLK)]
            for i in range(NBLK):
                ps = mid_ps.tile([128, H], F32)
                for k in range(HCH):
                    nc.tensor.matmul(out=ps[:], lhsT=ctxT[k][:, i * 128:(i + 1) * 128],
                                     rhs=Wo[k][:], start=(k == 0), stop=(k == HCH - 1))
                z = singles.tile([128, H], F32, name=f"z{i}", tag="zrow")
                nc.vector.tensor_add(z[:], ps[:], bob[:])
                nc.vector.tensor_add(z[:], z[:], hrows[i][:])
                _layernorm_rows(nc, singles, z, xrows[i], g1b, be1b, i, "ln1", epst)
            xT = [singles.tile([128, RPC], BF, name=f"xT{j}") for j in range(HCH)]
            for i in range(NBLK):
                for j in range(HCH):
                    tp = mid_ps.tile([128, 128], F32, name=f"tpx_{i}_{j}", tag="tp")
                    nc.tensor.transpose(out=tp[:], in_=xrows[i][:, j * 128:(j + 1) * 128],
                                        identity=ident[:])
                    nc.vector.tensor_copy(out=xT[j][:, i * 128:(i + 1) * 128], in_=tp[:])

            # FFN + LN2
            y1T = [singles.tile([128, RPC], BF, name=f"y1T{j}") for j in range(4)]
            for j in range(4):
                ps = mid_ps.tile([128, RPC], F32)
                for k in range(HCH):
                    nc.tensor.matmul(out=ps[:], lhsT=W1[k][:, j * 128:(j + 1) * 128],
                                     rhs=xT[k][:], start=(k == 0), stop=(k == HCH - 1))
                nc.scalar.activation(out=y1T[j][:], in_=ps[:], func=AF.Gelu,
                                     bias=b1c[:, j:j + 1], scale=1.0)
            for i in range(NBLK):
                ps = mid_ps.tile([128, H], F32)
                for k in range(4):
                    nc.tensor.matmul(out=ps[:], lhsT=y1T[k][:, i * 128:(i + 1) * 128],
                                     rhs=W2[k][:], start=(k == 0), stop=(k == 3))
                z = singles.tile([128, H], F32, name=f"z2{i}", tag="z2row")
                nc.vector.tensor_add(z[:], ps[:], b2b[:])
                nc.vector.tensor_add(z[:], z[:], xrows[i][:])
                o = singles.tile([128, H], F32, name=f"o{i}", tag="orow")
                _layernorm_rows(nc, singles, z, o, g2b, be2b, i, "ln2", epst)
                nc.sync.dma_start(out=out[i * 128:(i + 1) * 128, :], in_=o[:])
            ph6.__exit__(None, None, None)

    nc.compile()
    return nc


def _pair8(m):
    """[2*128, C] -> [128, 2*C] fp8, rows interleaved as DoubleRow halves."""
    r, c = m.shape
    assert r == 256
    return np.ascontiguousarray(
        m.reshape(2, 128, c).transpose(1, 0, 2).reshape(128, 2 * c)
    ).astype(ml_dtypes.float8_e4m3)


def build_in_maps(inputs):
    h = np.asarray(inputs["h"], np.float32)
    bf = ml_dtypes.bfloat16
    f8 = ml_dtypes.float8_e4m3
    hT = np.ascontiguousarray(h.T)

    src = np.asarray(inputs["edge_index"][0]).astype(np.int64)
    dst = np.asarray(inputs["edge_index"][1]).astype(np.int64)
    deg = np.bincount(dst, minlength=N).astype(np.float32)
    rec = 1.0 / np.maximum(deg, 1.0)
    w = (deg > 0).astype(np.float32)

    W = {k: np.asarray(inputs[k], np.float32) for k in
         ("W_src", "W_tgt", "Wq", "Wk", "Wv", "Wo", "W1", "W2")}
    B = {k: np.asarray(inputs[k], np.float32) for k in
         ("b_src", "b_tgt", "bq", "bk", "bv", "bo", "b1", "b2", "g1", "be1", "g2", "be2")}

    def bcast(v):
        return np.ascontiguousarray(np.tile(v[None, :], (128, 1)).astype(np.float32))

    # b_v shifts every v row by a constant, which shifts ctx by the same
    # constant (softmax weights sum to 1): fold it into b_o.
    bo_eff = B["bv"] @ W["Wo"] + B["bo"]

    common = {
        "hT8": _pair8(hT),
        "ws8": _pair8(W["W_src"]), "wt8": _pair8(W["W_tgt"]),
        "wq8": _pair8(W["Wq"]), "wk8": _pair8(W["Wk"]), "wv8": _pair8(W["Wv"]),
        "w_o": W["Wo"].astype(bf),
        "w_1": W["W1"].astype(bf), "w_2": W["W2"].astype(bf),
        "bst_c": np.ascontiguousarray(
            (B["b_src"] + B["b_tgt"]).reshape(HCH, 128).T.astype(np.float32)),
        "bq_c": np.ascontiguousarray(B["bq"].reshape(HCH, 128).T.astype(np.float32)),
        "bk_c": np.ascontiguousarray(B["bk"].reshape(HCH, 128).T.astype(np.float32)),
        "bo_b": bcast(bo_eff),
        "b1_c": np.ascontiguousarray(B["b1"].reshape(4, 128).T.astype(np.float32)),
        "b2_b": bcast(B["b2"]),
        "g1_b": bcast(B["g1"]),
        "be1_b": bcast(B["be1"]),
        "g2_b": bcast(B["g2"]),
        "be2_b": bcast(B["be2"]),
    }
    in_maps = []
    for c in range(NCORES):
        lo, hi = c * RPC, (c + 1) * RPC
        sel = (dst >= lo) & (dst < hi)
        idx = src[sel] * RPC + (dst[sel] - lo)
        a_t = np.bincount(idx, minlength=N * RPC).reshape(N, RPC)
        # [N, RPC] -> 16 tiles [128, 2, RPC], src chunks pair-interleaved
        at8 = a_t.reshape(16, 2, 128, RPC).transpose(0, 2, 1, 3).reshape(16 * 128, 2 * RPC)
        m = dict(common)
        m["at8"] = np.ascontiguousarray(at8).astype(f8)
        m["rw_b"] = bcast(np.concatenate([rec[lo:hi], w[lo:hi]]))
        m["h_rows"] = np.ascontiguousarray(h[lo:hi, :])
        m["hTo8"] = _pair8(np.ascontiguousarray(hT[:, lo:hi]))
        in_maps.append(m)
    return in_maps


def kernel(**inputs):
    if "prog" not in _CACHE:
        _CACHE["prog"] = _build_program()
    nc = _CACHE["prog"]
    in_maps = build_in_maps(inputs)
    res = run_bass_kernel_spmd(nc, in_maps, list(range(NCORES)))
    return np.concatenate([res.results[c]["out"] for c in range(NCORES)], axis=0)


if __name__ == "__main__":
    import reference
    inp = reference.setup_inputs()
    outp = kernel(**{k: np.asarray(v) for k, v in inp.items()})
    print("kernel out:", outp.shape, outp.dtype)
